# revision 4
# baseline (speedup 1.0000x reference)
"""AGCRN cell with per-node MLP-generated gate weights, on 8 TRN2 NeuronCores.

Math (reference):
    combined = adj @ concat([x, h], -1)          # [N, 257]
    cg = combined[nodes_ind]                     # [M, 257]
    gate(f, q, W, b) = einsum('ni,nd,dio->no', f, q, W) + q @ b
    r = sigmoid(gate(cg, q, W_r, b_r)); u = sigmoid(gate(cg, q, W_u, b_u))
    cn = [x_sel, r * h_sel]                      # [M, 257]
    cand = tanh(gate(cn, q, W_c, b_c))
    new_h = (1 - u) * (r * h_sel) + u * cand     # [M, 128]

Every output row depends only on its own (adj-row, q-row, x-row, h-row), so the
M rows shard cleanly across the 8 cores with W_*/C replicated — no collectives.

Per-core device graph (Mc = 256 rows, transposed orientation [feat, n]):
    cg^T [257, 256] = C^T @ A_sel^T       (PE, K=4096 tiled by 128)
    z^T[(d,i), n]   = q[n,d] * f^T[i, n]  (DVE, broadcast-AP multiply against a
                                           DMA-partition-broadcast q_bcast)
    gate^T [128, 256] = W_flat^T @ z^T + b^T @ q^T   (PE, K=8224 tiled by 128)
    sigmoid/tanh on ACT, final elementwise combine on DVE, f32 out.

The (d, i) contraction axis is reordered host-side as d*256 + i for i < 256
plus a 32-row tail (d, i=256) so K-tiles of 128 align with d-blocks; W_flat is
reordered to match. For gate c the feature order is [x cols 0..127, (r*h) cols
0..127, x col 128] so the tail is input-only (computable before r).
"""

import os
import sys

sys.path.insert(0, "/opt/trn_rl_repo")

import numpy as np
from ml_dtypes import bfloat16

import concourse.bass as bass
import concourse.tile as tile
from concourse import bacc, mybir
from concourse.bass_utils import run_bass_kernel_spmd

NC = 8
N = 4096
M = 2048
Mc = M // NC  # 256 rows per core
F = 257  # feature dim of concat([x, h])
FD = 288  # F padded: col 256 replicated 32x so the third m-group is [32, 256]
QD = 32
O = 128  # output dim
KT = 64  # full 128-row K-tiles of the (d, i<256) contraction
KROWS = QD * 256  # 8192
BF16 = mybir.dt.bfloat16
F32 = mybir.dt.float32

_COMPILED = None  # (nc, out_name) cache — compile once per process


def _build():
    nc = bacc.Bacc(
        "TRN2", target_bir_lowering=False, debug=False, num_devices=NC
    )
    d_AT = nc.dram_tensor("AT", [N, Mc], BF16, kind="ExternalInput").ap()
    d_C = nc.dram_tensor("C", [N, FD], BF16, kind="ExternalInput").ap()
    d_qT = nc.dram_tensor("qT", [QD, Mc], BF16, kind="ExternalInput").ap()
    d_qTf = nc.dram_tensor("qTflat", [1, KROWS], BF16, kind="ExternalInput").ap()
    d_xT = nc.dram_tensor("xT", [128, Mc], BF16, kind="ExternalInput").ap()
    d_xtail = nc.dram_tensor("xtail", [QD, Mc], BF16, kind="ExternalInput").ap()
    d_hT = nc.dram_tensor("hT", [128, Mc], F32, kind="ExternalInput").ap()
    d_W = {
        g: nc.dram_tensor(f"W{g}", [QD * 256 + QD, O], BF16, kind="ExternalInput").ap()
        for g in "ruc"
    }
    d_b = {
        g: nc.dram_tensor(f"b{g}", [QD, O], BF16, kind="ExternalInput").ap()
        for g in "ruc"
    }
    d_out = nc.dram_tensor("out", [O, Mc], F32, kind="ExternalOutput").ap()

    with tile.TileContext(nc) as tc:
        with (
            tc.tile_pool(name="res", bufs=1) as res,
            tc.tile_pool(name="pcg", bufs=1, space=bass.MemorySpace.PSUM) as pcg_pool,
            tc.tile_pool(name="pg", bufs=1, space=bass.MemorySpace.PSUM) as pg_pool,
        ):
            # ---- resident SBUF tiles + input DMAs ----
            C_sb = res.tile([128, 32 * FD], BF16, name="C_sb")
            nc.sync.dma_start(
                C_sb[:].rearrange("p (t c) -> p t c", c=FD),
                d_C.rearrange("(t p) c -> p t c", p=128),
            )
            AT_sb = res.tile([128, 32 * Mc], BF16, name="AT_sb")
            nc.sync.dma_start(
                AT_sb[:].rearrange("p (t n) -> p t n", n=Mc),
                d_AT.rearrange("(t p) n -> p t n", p=128),
            )
            W_sb, Wt_sb, b_sb = {}, {}, {}
            for g in "ruc":
                W_sb[g] = res.tile([128, KT * O], BF16, name=f"W{g}_sb")
                nc.sync.dma_start(
                    W_sb[g][:].rearrange("p (t m) -> p t m", m=O),
                    d_W[g][: KROWS].rearrange("(t p) m -> p t m", p=128),
                )
                Wt_sb[g] = res.tile([QD, O], BF16, name=f"W{g}t_sb")
                nc.sync.dma_start(Wt_sb[g][:], d_W[g][KROWS:])
                b_sb[g] = res.tile([QD, O], BF16, name=f"b{g}_sb")
                nc.sync.dma_start(b_sb[g][:], d_b[g][:])
            q_bc = res.tile([128, KROWS], BF16, name="q_bc")
            nc.sync.dma_start(q_bc[:], d_qTf.partition_broadcast(128))
            qT_sb = res.tile([QD, Mc], BF16, name="qT_sb")
            nc.sync.dma_start(qT_sb[:], d_qT[:])
            xT_sb = res.tile([128, Mc], BF16, name="xT_sb")
            nc.sync.dma_start(xT_sb[:], d_xT[:])
            xtail_sb = res.tile([QD, Mc], BF16, name="xtail_sb")
            nc.sync.dma_start(xtail_sb[:], d_xtail[:])
            hT_sb = res.tile([128, Mc], F32, name="hT_sb")
            nc.sync.dma_start(hT_sb[:], d_hT[:])

            # ---- phase 1: cg^T = C^T @ A_sel^T (three m-groups) ----
            pcg = [
                pcg_pool.tile([128, Mc], F32, name="pcg0"),
                pcg_pool.tile([128, Mc], F32, name="pcg1"),
                pcg_pool.tile([QD, Mc], F32, name="pcgt"),
            ]
            widths = [128, 128, QD]
            for gidx in range(3):
                for t in range(32):
                    nc.tensor.matmul(
                        pcg[gidx][:, :],
                        C_sb[:, t * FD + gidx * 128 : t * FD + gidx * 128 + widths[gidx]],
                        AT_sb[:, t * Mc : (t + 1) * Mc],
                        start=(t == 0),
                        stop=(t == 31),
                    )
            cgT = [
                res.tile([128, Mc], BF16, name="cgT0"),
                res.tile([128, Mc], BF16, name="cgT1"),
                res.tile([QD, Mc], BF16, name="cgTt"),
            ]
            for i in range(3):
                nc.scalar.activation(
                    cgT[i][:], pcg[i][:], mybir.ActivationFunctionType.Copy
                )

            # ---- phase 2: z_g^T via broadcast multiplies ----
            qb3 = q_bc[:].rearrange("p (a b) -> p a b", b=Mc)
            zg = [
                res.tile([128, KROWS], BF16, name="zg0"),
                res.tile([128, KROWS], BF16, name="zg1"),
            ]
            for ih in range(2):
                nc.vector.tensor_mul(
                    zg[ih][:].rearrange("p (a b) -> p a b", b=Mc),
                    cgT[ih][:].unsqueeze(1).broadcast_to((128, QD, Mc)),
                    qb3,
                )
            zgt = res.tile([QD, Mc], BF16, name="zgt")
            nc.vector.tensor_mul(zgt[:], qT_sb[:], cgT[2][:])

            # z_c^T input-only parts (no dependency on r)
            zc0 = res.tile([128, KROWS], BF16, name="zc0")
            nc.vector.tensor_mul(
                zc0[:].rearrange("p (a b) -> p a b", b=Mc),
                xT_sb[:].unsqueeze(1).broadcast_to((128, QD, Mc)),
                qb3,
            )
            zct = res.tile([QD, Mc], BF16, name="zct")
            nc.vector.tensor_mul(zct[:], qT_sb[:], xtail_sb[:])

            # ---- phase 3: gates r, u ----
            pr = pg_pool.tile([128, Mc], F32, name="pr")
            pu = pg_pool.tile([128, Mc], F32, name="pu")
            for t in range(KT):
                d, ih = t // 2, t % 2
                zsl = zg[ih][:, d * Mc : (d + 1) * Mc]
                wsl = slice(t * O, (t + 1) * O)
                nc.tensor.matmul(pr[:], W_sb["r"][:, wsl], zsl, start=(t == 0), stop=False)
                nc.tensor.matmul(pu[:], W_sb["u"][:, wsl], zsl, start=(t == 0), stop=False)
            nc.tensor.matmul(pr[:], Wt_sb["r"][:], zgt[:], start=False, stop=False)
            nc.tensor.matmul(pu[:], Wt_sb["u"][:], zgt[:], start=False, stop=False)
            nc.tensor.matmul(pr[:], b_sb["r"][:], qT_sb[:], start=False, stop=True)
            nc.tensor.matmul(pu[:], b_sb["u"][:], qT_sb[:], start=False, stop=True)

            r_sb = res.tile([128, Mc], F32, name="r_sb")
            u_sb = res.tile([128, Mc], F32, name="u_sb")
            nc.scalar.activation(r_sb[:], pr[:], mybir.ActivationFunctionType.Sigmoid)
            nc.scalar.activation(u_sb[:], pu[:], mybir.ActivationFunctionType.Sigmoid)

            # ---- phase 4: rh and z_c^T h-part ----
            rh_f = res.tile([128, Mc], F32, name="rh_f")
            nc.vector.tensor_mul(rh_f[:], r_sb[:], hT_sb[:])
            rh_b = res.tile([128, Mc], BF16, name="rh_b")
            nc.vector.tensor_copy(rh_b[:], rh_f[:])
            zc1 = res.tile([128, KROWS], BF16, name="zc1")
            nc.vector.tensor_mul(
                zc1[:].rearrange("p (a b) -> p a b", b=Mc),
                rh_b[:].unsqueeze(1).broadcast_to((128, QD, Mc)),
                qb3,
            )

            # ---- phase 5: gate c ----
            pc = pg_pool.tile([128, Mc], F32, name="pc")
            zc = [zc0, zc1]
            for t in range(KT):
                d, ih = t // 2, t % 2
                nc.tensor.matmul(
                    pc[:],
                    W_sb["c"][:, t * O : (t + 1) * O],
                    zc[ih][:, d * Mc : (d + 1) * Mc],
                    start=(t == 0),
                    stop=False,
                )
            nc.tensor.matmul(pc[:], Wt_sb["c"][:], zct[:], start=False, stop=False)
            nc.tensor.matmul(pc[:], b_sb["c"][:], qT_sb[:], start=False, stop=True)
            cand_sb = res.tile([128, Mc], F32, name="cand_sb")
            nc.scalar.activation(cand_sb[:], pc[:], mybir.ActivationFunctionType.Tanh)

            # ---- phase 6: new_h^T = rh + u * (cand - rh) ----
            t1 = res.tile([128, Mc], F32, name="t1")
            nc.vector.tensor_sub(t1[:], cand_sb[:], rh_f[:])
            t2 = res.tile([128, Mc], F32, name="t2")
            nc.vector.tensor_mul(t2[:], u_sb[:], t1[:])
            outT = res.tile([128, Mc], F32, name="outT")
            nc.vector.tensor_add(outT[:], rh_f[:], t2[:])
            nc.sync.dma_start(d_out[:], outT[:])

    nc.compile()
    return nc


def _get_compiled():
    global _COMPILED
    if _COMPILED is None:
        _COMPILED = _build()
    return _COMPILED


def _prep_inputs(x, h, query_vectors, adj, nodes_ind, W_u, b_u, W_r, b_r, W_c, b_c):
    idx = np.asarray(nodes_ind).astype(np.int64)
    f32 = np.float32
    x = np.asarray(x, f32)
    h = np.asarray(h, f32)
    q = np.asarray(query_vectors, f32)
    adj = np.asarray(adj, f32)

    C = np.concatenate([x, h], axis=1)  # [N, 257]
    C_dev = np.concatenate(
        [C[:, :256], np.repeat(C[:, 256:257], QD, axis=1)], axis=1
    ).astype(bfloat16)  # [N, 288]
    A_sel = adj[idx]  # [M, N]
    x_sel = x[idx]
    h_sel = h[idx]

    def flatW(W, perm):
        W = np.asarray(W, f32)[:, perm, :]  # [QD, 257, O]
        main = W[:, :256, :].reshape(QD * 256, O)
        tail = W[:, 256, :]
        return np.concatenate([main, tail], axis=0).astype(bfloat16)

    perm_id = list(range(F))
    perm_c = list(range(128)) + list(range(129, F)) + [128]
    Wf = {
        "r": flatW(W_r, perm_id),
        "u": flatW(W_u, perm_id),
        "c": flatW(W_c, perm_c),
    }
    bf = {
        "r": np.asarray(b_r, f32).astype(bfloat16),
        "u": np.asarray(b_u, f32).astype(bfloat16),
        "c": np.asarray(b_c, f32).astype(bfloat16),
    }

    in_maps = []
    for c in range(NC):
        sl = slice(c * Mc, (c + 1) * Mc)
        qT = np.ascontiguousarray(q[sl].T).astype(bfloat16)  # [32, 256]
        in_maps.append(
            {
                "AT": np.ascontiguousarray(A_sel[sl].T).astype(bfloat16),
                "C": C_dev,
                "qT": qT,
                "qTflat": qT.reshape(1, KROWS).copy(),
                "xT": np.ascontiguousarray(x_sel[sl, :128].T).astype(bfloat16),
                "xtail": np.broadcast_to(
                    x_sel[sl, 128], (QD, Mc)
                ).astype(bfloat16),
                "hT": np.ascontiguousarray(h_sel[sl].T).astype(f32),
                "Wr": Wf["r"],
                "Wu": Wf["u"],
                "Wc": Wf["c"],
                "br": bf["r"],
                "bu": bf["u"],
                "bc": bf["c"],
            }
        )
    return in_maps


def run(inputs: dict, trace: bool = False):
    nc = _get_compiled()
    in_maps = _prep_inputs(**inputs)
    res = run_bass_kernel_spmd(
        nc, in_maps, core_ids=list(range(NC)), trace=trace
    )
    shards = [res.results[c]["out"].T for c in range(NC)]  # each [256, 128]
    out = np.concatenate(shards, axis=0).astype(np.float32)  # [M, 128]
    return out, res


def kernel(**inputs) -> np.ndarray:
    out, _ = run(inputs, trace=bool(os.environ.get("BASS_KERNEL_TRACE")))
    return out


# revision 5
# speedup vs baseline: 1.3354x; 1.3354x over previous
"""AGCRN cell with per-node MLP-generated gate weights, on 8 TRN2 NeuronCores.

Math (reference):
    combined = adj @ concat([x, h], -1)          # [N, 257]
    cg = combined[nodes_ind]                     # [M, 257]
    gate(f, q, W, b) = einsum('ni,nd,dio->no', f, q, W) + q @ b
    r = sigmoid(gate(cg, q, W_r, b_r)); u = sigmoid(gate(cg, q, W_u, b_u))
    cn = [x_sel, r * h_sel]                      # [M, 257]
    cand = tanh(gate(cn, q, W_c, b_c))
    new_h = (1 - u) * (r * h_sel) + u * cand     # [M, 128]

Every output row depends only on its own (adj-row, q-row, x-row, h-row), so the
M rows shard cleanly across the 8 cores with W_*/C replicated — no collectives.

Per-core device graph (Mc = 256 rows, transposed orientation [feat, n]):
    cg^T [257, 256] = C^T @ A_sel^T       (PE, K=4096 tiled by 128)
    z^T[(d,i), n]   = q[n,d] * f^T[i, n]  (DVE/GpSimd broadcast-AP multiply
                                           against a partition-broadcast q_bcast)
    gate^T [128, 256] = W_flat^T @ z^T + b^T @ q^T   (PE, K=8224 tiled by 128)
    sigmoid/tanh on ACT, final elementwise combine on DVE, f32 out.

The (d, i) contraction axis is reordered host-side as d*256 + i for i < 256
plus a 32-row tail (d, i=256) so K-tiles of 128 align with d-blocks; W_flat is
reordered to match. For gate c the feature order is [x cols 0..127, (r*h) cols
0..127, x col 128] so the tail is input-only (computable before r).

All large DRAM tensors are pre-swizzled host-side into partition-major layout
([128, k_tiles*width], 4-16KB contiguous per partition) so every big DMA moves
large bursts; loads are chunked into separate tiles so matmuls start as soon
as their chunk lands. Input DMAs are spread over the sync/scalar HWDGE queues
plus the gpsimd SWDGE queue.
"""

import os
import sys

sys.path.insert(0, "/opt/trn_rl_repo")

import numpy as np
from ml_dtypes import bfloat16

import concourse.bass as bass
import concourse.tile as tile
from concourse import bacc, mybir
from concourse.bass_utils import run_bass_kernel_spmd

NC = 8
N = 4096
M = 2048
Mc = M // NC  # 256 rows per core
F = 257  # feature dim of concat([x, h])
FD = 288  # F padded: col 256 replicated 32x so the third m-group is [32, 256]
QD = 32
O = 128  # output dim
KT = 64  # full 128-row K-tiles of the (d, i<256) contraction
KROWS = QD * 256  # 8192
NCH = 4  # chunks for C/AT loads (8 k-tiles each)
ZCH = 4  # chunks for z construction (8 d's each)
BF16 = mybir.dt.bfloat16
F32 = mybir.dt.float32
AF = mybir.ActivationFunctionType

_COMPILED = None  # compile once per process


def _build():
    nc = bacc.Bacc("TRN2", target_bir_lowering=False, debug=False, num_devices=NC)
    d_AT = nc.dram_tensor("AT", [128, 32 * Mc], BF16, kind="ExternalInput").ap()
    d_C = nc.dram_tensor("C", [128, 32 * FD], BF16, kind="ExternalInput").ap()
    d_qT = nc.dram_tensor("qT", [QD, Mc], BF16, kind="ExternalInput").ap()
    d_qTf = nc.dram_tensor("qTflat", [1, KROWS], BF16, kind="ExternalInput").ap()
    d_xT = nc.dram_tensor("xT", [128, Mc], BF16, kind="ExternalInput").ap()
    d_xtail = nc.dram_tensor("xtail", [QD, Mc], BF16, kind="ExternalInput").ap()
    d_hT = nc.dram_tensor("hT", [128, Mc], F32, kind="ExternalInput").ap()
    d_W = {
        g: nc.dram_tensor(f"W{g}", [128, KT * O], BF16, kind="ExternalInput").ap()
        for g in "ruc"
    }
    d_Wt = {
        g: nc.dram_tensor(f"W{g}t", [QD, O], BF16, kind="ExternalInput").ap()
        for g in "ruc"
    }
    d_b = {
        g: nc.dram_tensor(f"b{g}", [QD, O], BF16, kind="ExternalInput").ap()
        for g in "ruc"
    }
    d_out = nc.dram_tensor("out", [O, Mc], F32, kind="ExternalOutput").ap()

    CW = 32 // NCH * FD  # C chunk width (8 k-tiles)
    AW = 32 // NCH * Mc  # AT chunk width
    WW = KT // 2 * O  # W half width (32 k-tiles = 16 d's)
    ZW = KROWS // ZCH  # z chunk width (8 d's)

    with tile.TileContext(nc) as tc:
        with (
            tc.tile_pool(name="res", bufs=1) as res,
            tc.tile_pool(name="psum", bufs=1, space=bass.MemorySpace.PSUM) as pp,
        ):
            # --- ACT table preload (sigmoid_and_others holds copy/sigmoid/tanh)
            warm = res.tile([1, 8], F32, name="warm")
            nc.vector.memset(warm[:], 0.0)
            warm2 = res.tile([1, 8], F32, name="warm2")
            nc.scalar.activation(warm2[:], warm[:], AF.Sigmoid)

            # --- small DMAs (sync queue) + q_bcast (gpsimd SWDGE)
            qT_sb = res.tile([QD, Mc], BF16, name="qT_sb")
            nc.sync.dma_start(qT_sb[:], d_qT[:])
            b_sb = {}
            for g in "ruc":
                b_sb[g] = res.tile([QD, O], BF16, name=f"b{g}_sb")
                nc.sync.dma_start(b_sb[g][:], d_b[g][:])
            xT_sb = res.tile([128, Mc], BF16, name="xT_sb")
            nc.sync.dma_start(xT_sb[:], d_xT[:])
            xtail_sb = res.tile([QD, Mc], BF16, name="xtail_sb")
            nc.sync.dma_start(xtail_sb[:], d_xtail[:])
            hT_sb = res.tile([128, Mc], F32, name="hT_sb")
            nc.sync.dma_start(hT_sb[:], d_hT[:])
            q_bc = res.tile([128, KROWS], BF16, name="q_bc")
            nc.gpsimd.dma_start(q_bc[:], d_qTf.partition_broadcast(128))
            Wt_sb = {}
            for g in "ruc":
                Wt_sb[g] = res.tile([QD, O], BF16, name=f"W{g}t_sb")
                nc.gpsimd.dma_start(Wt_sb[g][:], d_Wt[g][:])

            # --- chunked big loads: C/AT on sync, W halves on scalar
            C_sb, AT_sb = [], []
            for j in range(NCH):
                ct = res.tile([128, CW], BF16, name=f"C_sb{j}")
                nc.sync.dma_start(ct[:], d_C[:, j * CW : (j + 1) * CW])
                C_sb.append(ct)
                at = res.tile([128, AW], BF16, name=f"AT_sb{j}")
                nc.sync.dma_start(at[:], d_AT[:, j * AW : (j + 1) * AW])
                AT_sb.append(at)
            W_sb = {g: [None, None] for g in "ruc"}
            for half in range(2):
                for g in "ruc":
                    wt = res.tile([128, WW], BF16, name=f"W{g}_sb{half}")
                    nc.scalar.dma_start(wt[:], d_W[g][:, half * WW : (half + 1) * WW])
                    W_sb[g][half] = wt

            # --- gate bias matmuls first (warms PE, opens the psum groups)
            pr = pp.tile([128, Mc], F32, name="pr")
            pu = pp.tile([128, Mc], F32, name="pu")
            pc = pp.tile([128, Mc], F32, name="pc")
            nc.tensor.matmul(pr[:], b_sb["r"][:], qT_sb[:], start=True, stop=False)
            nc.tensor.matmul(pu[:], b_sb["u"][:], qT_sb[:], start=True, stop=False)
            nc.tensor.matmul(pc[:], b_sb["c"][:], qT_sb[:], start=True, stop=False)

            # --- phase 1: cg^T = C^T @ A_sel^T (three m-groups, m-group major)
            pcg = [
                pp.tile([128, Mc], F32, name="pcg0"),
                pp.tile([128, Mc], F32, name="pcg1"),
                pp.tile([QD, Mc], F32, name="pcgt"),
            ]
            widths = [128, 128, QD]
            for gi in range(3):
                for t in range(32):
                    j, tt = t // 8, t % 8
                    nc.tensor.matmul(
                        pcg[gi][:, :],
                        C_sb[j][:, tt * FD + gi * 128 : tt * FD + gi * 128 + widths[gi]],
                        AT_sb[j][:, tt * Mc : (tt + 1) * Mc],
                        start=(t == 0),
                        stop=(t == 31),
                    )
            cgT = [
                res.tile([128, Mc], BF16, name="cgT0"),
                res.tile([128, Mc], BF16, name="cgT1"),
                res.tile([QD, Mc], BF16, name="cgTt"),
            ]
            for i in range(3):
                nc.scalar.activation(cgT[i][:], pcg[i][:], AF.Copy)

            # --- phase 2: z_g^T chunks (DVE), z_c^T input-only parts (GpSimd)
            def zchunks(dst_list, src_ap, engine, nparts=128):
                # dst[k][p, dd*Mc + n] = src[p, n] * q[n, d0+dd]
                for k in range(ZCH):
                    zt = dst_list[k]
                    engine.tensor_mul(
                        zt[:].rearrange("p (a b) -> p a b", b=Mc),
                        src_ap.unsqueeze(1).broadcast_to((nparts, QD // ZCH, Mc)),
                        q_bc[:nparts, k * ZW : (k + 1) * ZW].rearrange(
                            "p (a b) -> p a b", b=Mc
                        ),
                    )

            zg0 = [res.tile([128, ZW], BF16, name=f"zg0_{k}") for k in range(ZCH)]
            zg1 = [res.tile([128, ZW], BF16, name=f"zg1_{k}") for k in range(ZCH)]
            zchunks(zg0, cgT[0][:], nc.vector)
            zchunks(zg1, cgT[1][:], nc.vector)
            zgt = res.tile([QD, Mc], BF16, name="zgt")
            nc.vector.tensor_mul(zgt[:], qT_sb[:], cgT[2][:])
            zc0 = [res.tile([128, ZW], BF16, name=f"zc0_{k}") for k in range(ZCH)]
            zchunks(zc0, xT_sb[:], nc.gpsimd)
            zct = res.tile([QD, Mc], BF16, name="zct")
            nc.gpsimd.tensor_mul(zct[:], qT_sb[:], xtail_sb[:])

            # --- phase 3: gates r, u (k-tile t -> d = t//2, ihalf = t%2)
            def zslice(zl, d):
                return zl[d // (QD // ZCH)][:, (d % (QD // ZCH)) * Mc : (d % (QD // ZCH) + 1) * Mc]

            for t in range(KT):
                d, ih = t // 2, t % 2
                zsl = zslice(zg0 if ih == 0 else zg1, d)
                wsl = slice((t % 32) * O, (t % 32 + 1) * O)
                nc.tensor.matmul(pr[:], W_sb["r"][t // 32][:, wsl], zsl, start=False, stop=False)
                nc.tensor.matmul(pu[:], W_sb["u"][t // 32][:, wsl], zsl, start=False, stop=False)
            nc.tensor.matmul(pr[:], Wt_sb["r"][:], zgt[:], start=False, stop=True)
            nc.tensor.matmul(pu[:], Wt_sb["u"][:], zgt[:], start=False, stop=True)

            r_sb = res.tile([128, Mc], F32, name="r_sb")
            nc.scalar.activation(r_sb[:], pr[:], AF.Sigmoid)
            u_sb = res.tile([128, Mc], F32, name="u_sb")
            nc.scalar.activation(u_sb[:], pu[:], AF.Sigmoid)

            # --- gate c, x-part (fills PE while sigmoid/rh/zc1 run)
            for t in range(0, KT, 2):  # ihalf = 0 tiles
                d = t // 2
                nc.tensor.matmul(
                    pc[:],
                    W_sb["c"][t // 32][:, (t % 32) * O : (t % 32 + 1) * O],
                    zslice(zc0, d),
                    start=False,
                    stop=False,
                )
            nc.tensor.matmul(pc[:], Wt_sb["c"][:], zct[:], start=False, stop=False)

            # --- phase 4: rh and z_c^T h-part (chunked for early c matmuls)
            rh_f = res.tile([128, Mc], F32, name="rh_f")
            nc.vector.tensor_mul(rh_f[:], r_sb[:], hT_sb[:])
            rh_b = res.tile([128, Mc], BF16, name="rh_b")
            nc.vector.tensor_copy(rh_b[:], rh_f[:])
            zc1 = [res.tile([128, ZW], BF16, name=f"zc1_{k}") for k in range(ZCH)]
            zchunks(zc1, rh_b[:], nc.vector)

            # --- phase 5: gate c h-part + tanh
            for t in range(1, KT, 2):  # ihalf = 1 tiles
                d = t // 2
                nc.tensor.matmul(
                    pc[:],
                    W_sb["c"][t // 32][:, (t % 32) * O : (t % 32 + 1) * O],
                    zslice(zc1, d),
                    start=False,
                    stop=(t == KT - 1),
                )
            cand_sb = res.tile([128, Mc], F32, name="cand_sb")
            nc.scalar.activation(cand_sb[:], pc[:], AF.Tanh)

            # --- phase 6: new_h^T = rh + u * (cand - rh)
            t1 = res.tile([128, Mc], F32, name="t1")
            nc.vector.tensor_sub(t1[:], cand_sb[:], rh_f[:])
            t2 = res.tile([128, Mc], F32, name="t2")
            nc.vector.tensor_mul(t2[:], u_sb[:], t1[:])
            outT = res.tile([128, Mc], F32, name="outT")
            nc.vector.tensor_add(outT[:], rh_f[:], t2[:])
            nc.sync.dma_start(d_out[:], outT[:])

    nc.compile()
    return nc


def _get_compiled():
    global _COMPILED
    if _COMPILED is None:
        _COMPILED = _build()
    return _COMPILED


def _pmajor(a, width):
    """[n_tiles*128, width] row-major -> [128, n_tiles*width] partition-major."""
    nt = a.shape[0] // 128
    return np.ascontiguousarray(
        a.reshape(nt, 128, width).transpose(1, 0, 2).reshape(128, nt * width)
    )


def _prep_inputs(x, h, query_vectors, adj, nodes_ind, W_u, b_u, W_r, b_r, W_c, b_c):
    idx = np.asarray(nodes_ind).astype(np.int64)
    f32 = np.float32
    x = np.asarray(x, f32)
    h = np.asarray(h, f32)
    q = np.asarray(query_vectors, f32)
    adj = np.asarray(adj, f32)

    C = np.concatenate([x, h], axis=1)  # [N, 257]
    C_dev = np.concatenate(
        [C[:, :256], np.repeat(C[:, 256:257], QD, axis=1)], axis=1
    ).astype(bfloat16)  # [N, 288]
    C_pm = _pmajor(C_dev, FD)  # [128, 32*288]
    A_sel = adj[idx]  # [M, N]
    x_sel = x[idx]
    h_sel = h[idx]

    def flatW(W, perm):
        W = np.asarray(W, f32)[:, perm, :]  # [QD, 257, O]
        main = W[:, :256, :].reshape(QD * 256, O).astype(bfloat16)
        tail = np.ascontiguousarray(W[:, 256, :]).astype(bfloat16)
        return _pmajor(main, O), tail  # [128, 64*128], [32, 128]

    perm_id = list(range(F))
    perm_c = list(range(128)) + list(range(129, F)) + [128]
    Wf = {"r": flatW(W_r, perm_id), "u": flatW(W_u, perm_id), "c": flatW(W_c, perm_c)}
    bf = {
        "r": np.asarray(b_r, f32).astype(bfloat16),
        "u": np.asarray(b_u, f32).astype(bfloat16),
        "c": np.asarray(b_c, f32).astype(bfloat16),
    }

    in_maps = []
    for c in range(NC):
        sl = slice(c * Mc, (c + 1) * Mc)
        qT = np.ascontiguousarray(q[sl].T).astype(bfloat16)  # [32, 256]
        AT = np.ascontiguousarray(A_sel[sl].T).astype(bfloat16)  # [4096, 256]
        in_maps.append(
            {
                "AT": _pmajor(AT, Mc),
                "C": C_pm,
                "qT": qT,
                "qTflat": qT.reshape(1, KROWS).copy(),
                "xT": np.ascontiguousarray(x_sel[sl, :128].T).astype(bfloat16),
                "xtail": np.broadcast_to(x_sel[sl, 128], (QD, Mc)).astype(bfloat16),
                "hT": np.ascontiguousarray(h_sel[sl].T).astype(f32),
                "Wr": Wf["r"][0],
                "Wu": Wf["u"][0],
                "Wc": Wf["c"][0],
                "Wrt": Wf["r"][1],
                "Wut": Wf["u"][1],
                "Wct": Wf["c"][1],
                "br": bf["r"],
                "bu": bf["u"],
                "bc": bf["c"],
            }
        )
    return in_maps


def run(inputs: dict, trace: bool = False):
    nc = _get_compiled()
    in_maps = _prep_inputs(**inputs)
    res = run_bass_kernel_spmd(nc, in_maps, core_ids=list(range(NC)), trace=trace)
    shards = [res.results[c]["out"].T for c in range(NC)]  # each [256, 128]
    out = np.concatenate(shards, axis=0).astype(np.float32)  # [M, 128]
    return out, res


def kernel(**inputs) -> np.ndarray:
    out, _ = run(inputs, trace=bool(os.environ.get("BASS_KERNEL_TRACE")))
    return out
